# revision 8
# baseline (speedup 1.0000x reference)
"""Trainium2 Bass kernel for CTCDecoder-like module.

Reference computes (per batch b, with A = x[:, b, :] of shape (L, D)):
    wx     = A @ Ww^T + Wb
    scores = A @ wx^T                       # (L, L) -- never materialized here
    y      = scores @ A
    logits = y @ Lw^T + Lb
    out    = log_softmax(logits, axis=-1)

Algebraic collapse used by this kernel (exact in real arithmetic):
    scores = A Ww A^T + (A Wb) 1^T
    y      = A Ww (A^T A) + (A Wb)(1^T A) = A (Ww G + Wb c^T) = A H
    logits = A (H Lw^T) + 1 Lb^T = A Mt + 1 Lb^T
with G = A^T A (D x D Gram), c = A^T 1 (column sums), H = Ww G + Wb c^T,
Mt = H Lw^T (D x V).  The (L, L) score matrix is never formed.

Sharding: 8 cores = 4 batches x 2 halves of L.  Each core computes its
batch's Gram redundantly (pure SPMD, no collectives), then its own
2048-token logits + log_softmax.  All matmuls run as float32r (full-rate
fp32 streaming on TRN2 for free dims >= 256), accumulated in fp32 PSUM.
"""

import numpy as np

L, B, D, V = 4096, 4, 512, 1000
P = 128
NCORES = 8
LC = L // 2            # tokens per core
NKT = L // P           # 32 k-tiles over full L (Gram)
NLT = LC // P          # 16 l-tiles per core
NDT = D // P           # 4 d-tiles
NV = 2
VW = V // NV           # 500

_CACHED_NC = None
_ONES_COL = np.ones((P, 1), np.float32)
_ONES_MAT = np.ones((P, P), np.float32)


def _build_nc():
    import concourse.bass as bass
    import concourse.tile as tile
    import concourse.mybir as mybir
    from concourse import bacc
    from concourse.bass import ds, ts

    f32 = mybir.dt.float32
    f32r = mybir.dt.float32r
    bf16 = mybir.dt.bfloat16
    EXP = mybir.ActivationFunctionType.Exp
    LN = mybir.ActivationFunctionType.Ln
    IDENT = mybir.ActivationFunctionType.Identity
    X = mybir.AxisListType.X

    # Serve Exp/Ln/Identity/Copy from the single act-func table that
    # contains them all ("natural_log_exp_and_others"), instead of letting
    # the table-load pass bounce between per-func tables (1283 ns per
    # reload on ScalarE).  Order (and thus act_func_set_id) is preserved;
    # only the advertised contents of the other tables are hidden.
    import concourse.bacc as bacc_mod
    from concourse.hw_specs import get_activation_tables

    def _pinned_tables(arch, _orig=get_activation_tables):
        tables = _orig(arch)
        keep = "natural_log_exp_and_others"
        if keep in tables:
            tables = {
                name: (funcs if name == keep else set())
                for name, funcs in tables.items()
            }
        return tables

    bacc_mod.get_activation_tables = _pinned_tables

    nc = bacc.Bacc("TRN2", target_bir_lowering=False, debug=False, num_devices=NCORES)

    a_dram = nc.dram_tensor("a_full", (NLT, P, D), f32r, kind="ExternalInput")
    at_dram = nc.dram_tensor("at", (P, NDT, LC), f32r, kind="ExternalInput")
    wwt_dram = nc.dram_tensor("wwt", (P, NDT, D), f32r, kind="ExternalInput")
    lwt_dram = nc.dram_tensor("lwt", (P, NDT, V), f32r, kind="ExternalInput")
    wb_dram = nc.dram_tensor("wb", (1, D), f32r, kind="ExternalInput")
    lb_dram = nc.dram_tensor("lb", (P, V), f32r, kind="ExternalInput")
    onesc_dram = nc.dram_tensor("ones_col", (P, 1), f32r, kind="ExternalInput")
    onesr_dram = nc.dram_tensor("ones_mat", (P, P), f32r, kind="ExternalInput")
    out_dram = nc.dram_tensor("out", (NLT, P, V), f32, kind="ExternalOutput")

    with tile.TileContext(nc) as tc:
        with (
            tc.tile_pool(name="const", bufs=1) as const,
            tc.tile_pool(name="big", bufs=1) as big,
            tc.tile_pool(name="astream", bufs=8) as astream,
            tc.tile_pool(name="work", bufs=3) as work,
            tc.tile_pool(name="stat", bufs=4) as stat,
            tc.tile_pool(name="ps", bufs=1, space="PSUM") as ps,
            tc.tile_pool(name="dram", bufs=1, space="DRAM") as dram,
        ):
            # ---- tiny constants first (ones_col gates the first ct matmul) ----
            ones_col = const.tile([P, 1], f32r, name="ones_col", tag="ones_col")
            nc.sync.dma_start(ones_col, onesc_dram[:])
            ones_mat = const.tile([P, P], f32r, name="ones_mat", tag="ones_mat")
            nc.sync.dma_start(ones_mat, onesr_dram[:])
            wb_sb = const.tile([1, D], f32r, name="wb_sb", tag="wb_sb")
            nc.sync.dma_start(wb_sb, wb_dram[:])
            lb_sb = const.tile([P, V], f32r, name="lb_sb", tag="lb_sb")
            nc.sync.dma_start(lb_sb, lb_dram[:])
            at_sb = big.tile([P, NDT, LC], f32r, name="at_sb", tag="at_sb")
            wwt_sb = big.tile([P, NDT, D], f32r, name="wwt_sb", tag="wwt_sb")
            lwt_sb = big.tile([P, NDT, V], f32r, name="lwt_sb", tag="lwt_sb")

            # ---- phase 1: Gram G = A^T A and c^T = 1^T A over full L ----
            g_ps = [
                ps.tile([P, D], f32, name=f"g_ps{mt}", tag=f"b{mt}")
                for mt in range(NDT)
            ]
            ct_ps = ps.tile([1, D], f32, name="ct_ps", tag="b4")
            for kt in range(NLT):
                a_t = astream.tile([P, D], f32r, name="a_t", tag="a_t")
                nc.sync.dma_start(a_t, a_dram[kt])
                first, last = kt == 0, kt == NLT - 1
                for mt in range(NDT):
                    nc.tensor.matmul(
                        g_ps[mt], a_t[:, ts(mt, P)], a_t,
                        start=first, stop=last,
                    )
                nc.tensor.matmul(
                    ct_ps, ones_col, a_t, start=first, stop=last,
                )

            # resident operands stream in while the PE chews on the Gram
            nc.sync.dma_start(wwt_sb, wwt_dram[:])
            nc.sync.dma_start(lwt_sb, lwt_dram[:])
            nc.sync.dma_start(at_sb, at_dram[:])

            g_sb = big.tile([P, NDT, D], f32r, name="g_sb", tag="g_sb")
            for mt in range(NDT):
                nc.vector.tensor_copy(g_sb[:, mt, :], g_ps[mt])
            c_row = big.tile([1, D], f32r, name="c_row", tag="c_row")
            nc.vector.tensor_copy(c_row, ct_ps)

            # ---- pair AllReduce of [G | c] (each core has half-L partials) ----
            ccin = dram.tile([NDT * P + 1, D], f32, name="ccin", tag="ccin")
            ccout = dram.tile([NDT * P + 1, D], f32, name="ccout", tag="ccout")
            nc.sync.dma_start(
                ccin[0:NDT * P, :].rearrange("(kt p) d -> p kt d", p=P),
                g_sb[:].bitcast(f32),
            )
            nc.sync.dma_start(ccin[NDT * P:NDT * P + 1, :], c_row.bitcast(f32))
            nc.gpsimd.collective_compute(
                "AllReduce",
                mybir.AluOpType.add,
                replica_groups=[[0, 1], [2, 3], [4, 5], [6, 7]],
                ins=[ccin[:].opt()],
                outs=[ccout[:].opt()],
            )
            nc.sync.dma_start(
                g_sb,
                ccout[0:NDT * P, :].rearrange("(kt p) d -> p kt d", p=P).bitcast(f32r),
            )
            nc.sync.dma_start(c_row, ccout[NDT * P:NDT * P + 1, :].bitcast(f32r))

            # ---- phase 2: Ht = G Ww^T + c (x) Wb ; Mt = Ht^T Lw^T ----
            ht_sb = big.tile([P, NDT, D], f32r, name="ht_sb", tag="ht_sb")
            for jt in range(NDT):
                hp = ps.tile([P, D], f32, name=f"hp{jt}", tag=f"b{5 + jt % 3}")
                for kt in range(NDT):
                    nc.tensor.matmul(
                        hp, g_sb[:, kt, ts(jt, P)], wwt_sb[:, kt, :],
                        start=(kt == 0), stop=False,
                    )
                nc.tensor.matmul(
                    hp, c_row[:, ts(jt, P)], wb_sb, start=False, stop=True,
                )
                nc.vector.tensor_copy(ht_sb[:, jt, :], hp)

            mt_sb = big.tile([P, NDT, V], f32r, name="mt_sb", tag="mt_sb")
            for dt in range(NDT):
                for nt in range(NV):
                    mp = ps.tile(
                        [P, VW], f32, name=f"mp{dt}_{nt}",
                        tag=f"b{(dt * NV + nt) % 8}",
                    )
                    for jt in range(NDT):
                        nc.tensor.matmul(
                            mp,
                            ht_sb[:, jt, ts(dt, P)],
                            lwt_sb[:, jt, ds(nt * VW, VW)],
                            start=(jt == 0), stop=(jt == NDT - 1),
                        )
                    nc.vector.tensor_copy(mt_sb[:, dt, ds(nt * VW, VW)], mp)

            # ---- phase 3: logits = A Mt + 1 Lb^T, then log_softmax rows ----
            for lt in range(NLT):
                lp = []
                for nt in range(NV):
                    p_t = ps.tile(
                        [P, VW], f32, name=f"lp{lt}_{nt}",
                        tag=f"b{(lt * NV + nt) % 8}",
                    )
                    for kt in range(NDT):
                        nc.tensor.matmul(
                            p_t,
                            at_sb[:, kt, ts(lt, P)],
                            mt_sb[:, kt, ds(nt * VW, VW)],
                            start=(kt == 0), stop=False,
                        )
                    nc.tensor.matmul(
                        p_t, ones_mat, lb_sb[:, ds(nt * VW, VW)],
                        start=False, stop=True,
                    )
                    lp.append(p_t)

                mx0 = stat.tile([P, 1], f32, name="mx0", tag="mx0")
                nc.vector.reduce_max(mx0, lp[0], axis=X)
                mx1 = stat.tile([P, 1], f32, name="mx1", tag="mx1")
                nc.vector.reduce_max(mx1, lp[1], axis=X)
                nmx = stat.tile([P, 1], f32, name="nmx", tag="nmx")
                nc.vector.tensor_tensor(nmx, mx0, mx1, op=mybir.AluOpType.max)
                nc.vector.tensor_scalar_mul(nmx, nmx, -1.0)

                esc = work.tile([P, V], f32, name="esc", tag="esc")
                se0 = stat.tile([P, 1], f32, name="se0", tag="se0")
                se1 = stat.tile([P, 1], f32, name="se1", tag="se1")
                nc.scalar.activation(
                    esc[:, 0:VW], lp[0], EXP, bias=nmx, scale=1.0, accum_out=se0
                )
                nc.scalar.activation(
                    esc[:, VW:V], lp[1], EXP, bias=nmx, scale=1.0, accum_out=se1
                )
                sume = stat.tile([P, 1], f32, name="sume", tag="sume")
                nc.vector.tensor_add(sume, se0, se1)
                lns = stat.tile([P, 1], f32, name="lns", tag="lns")
                nc.scalar.activation(lns, sume, LN)
                shift = stat.tile([P, 1], f32, name="shift", tag="shift")
                nc.vector.tensor_tensor(
                    shift, nmx, lns, op=mybir.AluOpType.subtract
                )

                out_sb = work.tile([P, V], f32, name="out_sb", tag="out_sb")
                nc.scalar.activation(
                    out_sb[:, 0:VW], lp[0], IDENT, bias=shift, scale=1.0
                )
                nc.vector.tensor_scalar_add(out_sb[:, VW:V], lp[1], shift)
                nc.sync.dma_start(out_dram[lt], out_sb)

    nc.compile()
    return nc


def _get_nc():
    global _CACHED_NC
    if _CACHED_NC is None:
        _CACHED_NC = _build_nc()
    return _CACHED_NC


def _make_in_maps(x, Ww, Wb, Lw, Lb):
    x = np.asarray(x, dtype=np.float32)
    Ww = np.asarray(Ww, dtype=np.float32)
    Wb = np.asarray(Wb, dtype=np.float32)
    Lw = np.asarray(Lw, dtype=np.float32)
    Lb = np.asarray(Lb, dtype=np.float32)

    wwt = np.ascontiguousarray(
        Ww.T.reshape(NDT, P, D).transpose(1, 0, 2)
    )  # (P, NDT, D)
    lwt = np.ascontiguousarray(
        Lw.T.reshape(NDT, P, V).transpose(1, 0, 2)
    )  # (P, NDT, V)
    wb = np.ascontiguousarray(Wb.reshape(1, D))
    lb = np.zeros((P, V), np.float32)
    lb[0, :] = Lb

    in_maps = []
    for core in range(NCORES):
        b, h = core // 2, core % 2
        a_b = np.ascontiguousarray(x[:, b, :])                 # (L, D)
        a_full = np.ascontiguousarray(a_b[h * LC:(h + 1) * LC].reshape(NLT, P, D))
        at = np.ascontiguousarray(
            a_b[h * LC:(h + 1) * LC, :].T.reshape(NDT, P, LC).transpose(1, 0, 2)
        )  # (P, NDT, LC)
        in_maps.append({
            "a_full": np.ascontiguousarray(a_full),
            "at": at,
            "wwt": wwt,
            "lwt": lwt,
            "wb": wb,
            "lb": lb,
            "ones_col": _ONES_COL,
            "ones_mat": _ONES_MAT,
        })
    return in_maps


def kernel(x, Ww, Wb, Lw, Lb, _trace=False):
    from concourse.bass_utils import run_bass_kernel_spmd

    nc = _get_nc()
    in_maps = _make_in_maps(x, Ww, Wb, Lw, Lb)
    res = run_bass_kernel_spmd(
        nc, in_maps, core_ids=list(range(NCORES)), trace=_trace
    )
    out = np.empty((L, B, V), np.float32)
    for core in range(NCORES):
        b, h = core // 2, core % 2
        out[h * LC:(h + 1) * LC, b, :] = (
            res.results[core]["out"].reshape(LC, V)
        )
    if _trace:
        kernel._last_results = res
    return out


# revision 10
# speedup vs baseline: 1.4835x; 1.4835x over previous
"""Trainium2 Bass kernel for CTCDecoder-like module.

Reference computes (per batch b, with A = x[:, b, :] of shape (L, D)):
    wx     = A @ Ww^T + Wb
    scores = A @ wx^T                       # (L, L) -- never materialized here
    y      = scores @ A
    logits = y @ Lw^T + Lb
    out    = log_softmax(logits, axis=-1)

Algebraic collapse used by this kernel (exact in real arithmetic):
    scores = A Ww A^T + (A Wb) 1^T
    y      = A Ww (A^T A) + (A Wb)(1^T A) = A (Ww G + Wb c^T) = A H
    logits = A (H Lw^T) + 1 Lb^T = A Mt + 1 Lb^T
with G = A^T A (D x D Gram), c = A^T 1 (column sums), H = Ww G + Wb c^T,
Mt = H Lw^T (D x V).  The (L, L) score matrix is never formed.

Sharding: 8 cores = 4 batches x 2 halves of L.  Each core computes its
batch's Gram redundantly (pure SPMD, no collectives), then its own
2048-token logits + log_softmax.  All matmuls run as float32r (full-rate
fp32 streaming on TRN2 for free dims >= 256), accumulated in fp32 PSUM.
"""

import numpy as np

L, B, D, V = 4096, 4, 512, 1000
P = 128
NCORES = 8
LC = L // 2            # tokens per core
NKT = L // P           # 32 k-tiles over full L (Gram)
NLT = LC // P          # 16 l-tiles per core
NDT = D // P           # 4 d-tiles
NV = 2
VW = V // NV           # 500

_CACHED_NC = None
_ONES_COL = np.ones((P, 1), np.float32)
_ONES_MAT = np.ones((P, P), np.float32)


def _build_nc():
    import concourse.bass as bass
    import concourse.tile as tile
    import concourse.mybir as mybir
    from concourse import bacc
    from concourse.bass import ds, ts

    f32 = mybir.dt.float32
    f32r = mybir.dt.float32r
    bf16 = mybir.dt.bfloat16
    EXP = mybir.ActivationFunctionType.Exp
    LN = mybir.ActivationFunctionType.Ln
    IDENT = mybir.ActivationFunctionType.Identity
    X = mybir.AxisListType.X

    # Serve Exp/Ln/Identity/Copy from the single act-func table that
    # contains them all ("natural_log_exp_and_others"), instead of letting
    # the table-load pass bounce between per-func tables (1283 ns per
    # reload on ScalarE).  Order (and thus act_func_set_id) is preserved;
    # only the advertised contents of the other tables are hidden.
    import concourse.bacc as bacc_mod
    from concourse.hw_specs import get_activation_tables

    def _pinned_tables(arch, _orig=get_activation_tables):
        tables = _orig(arch)
        keep = "natural_log_exp_and_others"
        if keep in tables:
            tables = {
                name: (funcs if name == keep else set())
                for name, funcs in tables.items()
            }
        return tables

    bacc_mod.get_activation_tables = _pinned_tables

    nc = bacc.Bacc("TRN2", target_bir_lowering=False, debug=False)

    a_dram = nc.dram_tensor("a_full", (NKT, P, D), f32r, kind="ExternalInput")
    at_dram = nc.dram_tensor("at", (P, NDT, LC), f32r, kind="ExternalInput")
    wwt_dram = nc.dram_tensor("wwt", (P, NDT, D), f32r, kind="ExternalInput")
    lwt_dram = nc.dram_tensor("lwt", (P, NDT, V), f32r, kind="ExternalInput")
    wb_dram = nc.dram_tensor("wb", (1, D), f32r, kind="ExternalInput")
    lb_dram = nc.dram_tensor("lb", (P, V), f32r, kind="ExternalInput")
    onesc_dram = nc.dram_tensor("ones_col", (P, 1), f32r, kind="ExternalInput")
    onesr_dram = nc.dram_tensor("ones_mat", (P, P), f32r, kind="ExternalInput")
    out_dram = nc.dram_tensor("out", (NLT, P, V), f32, kind="ExternalOutput")

    with tile.TileContext(nc) as tc:
        with (
            tc.tile_pool(name="const", bufs=1) as const,
            tc.tile_pool(name="big", bufs=1) as big,
            tc.tile_pool(name="astream", bufs=8) as astream,
            tc.tile_pool(name="work", bufs=3) as work,
            tc.tile_pool(name="stat", bufs=4) as stat,
            tc.tile_pool(name="ps", bufs=1, space="PSUM") as ps,
        ):
            # ---- tiny constants first (ones_col gates the first ct matmul) ----
            ones_col = const.tile([P, 1], f32r, name="ones_col", tag="ones_col")
            nc.sync.dma_start(ones_col, onesc_dram[:])
            ones_mat = const.tile([P, P], f32r, name="ones_mat", tag="ones_mat")
            wb_sb = const.tile([1, D], f32r, name="wb_sb", tag="wb_sb")
            lb_sb = const.tile([P, V], f32r, name="lb_sb", tag="lb_sb")
            at_sb = big.tile([P, NDT, LC], f32r, name="at_sb", tag="at_sb")
            wwt_sb = big.tile([P, NDT, D], f32r, name="wwt_sb", tag="wwt_sb")
            lwt_sb = big.tile([P, NDT, V], f32r, name="lwt_sb", tag="lwt_sb")

            # ---- phase 1: Gram G = A^T A and c^T = 1^T A over full L ----
            g_ps = [
                ps.tile([P, D], f32, name=f"g_ps{mt}", tag=f"b{mt}")
                for mt in range(NDT)
            ]
            ct_ps = ps.tile([1, D], f32, name="ct_ps", tag="b4")
            acc = big.tile([P, D], f32, name="acc", tag="acc")
            nc.vector.memset(acc, 0.0)
            for kt in range(NKT):
                a_t = astream.tile([P, D], f32r, name="a_t", tag="a_t")
                nc.sync.dma_start(a_t, a_dram[kt])
                first, last = kt == 0, kt == NKT - 1
                for mt in range(NDT):
                    nc.tensor.matmul(
                        g_ps[mt], a_t[:, ts(mt, P)], a_t,
                        start=first, stop=last,
                    )
                nc.vector.tensor_add(acc, acc, a_t.bitcast(f32))
            acc_r = big.tile([P, D], f32r, name="acc_r", tag="acc_r")
            nc.vector.tensor_copy(acc_r, acc)
            nc.tensor.matmul(ct_ps, ones_col, acc_r, start=True, stop=True)

            # resident operands stream in while the PE chews on the Gram
            nc.sync.dma_start(wb_sb, wb_dram[:])
            nc.sync.dma_start(wwt_sb, wwt_dram[:])
            nc.sync.dma_start(lb_sb, lb_dram[:])
            nc.sync.dma_start(ones_mat, onesr_dram[:])
            nc.sync.dma_start(lwt_sb, lwt_dram[:])
            nc.sync.dma_start(at_sb, at_dram[:])

            g_sb = big.tile([P, NDT, D], f32r, name="g_sb", tag="g_sb")
            for mt in range(NDT):
                nc.vector.tensor_copy(g_sb[:, mt, :], g_ps[mt])
            c_row = big.tile([1, D], f32r, name="c_row", tag="c_row")
            nc.vector.tensor_copy(c_row, ct_ps)

            # ---- phase 2: Ht = G Ww^T + c (x) Wb ; Mt = Ht^T Lw^T ----
            ht_sb = big.tile([P, NDT, D], f32r, name="ht_sb", tag="ht_sb")
            for jt in range(NDT):
                hp = ps.tile([P, D], f32, name=f"hp{jt}", tag=f"b{5 + jt % 3}")
                for kt in range(NDT):
                    nc.tensor.matmul(
                        hp, g_sb[:, kt, ts(jt, P)], wwt_sb[:, kt, :],
                        start=(kt == 0), stop=False,
                    )
                nc.tensor.matmul(
                    hp, c_row[:, ts(jt, P)], wb_sb, start=False, stop=True,
                )
                nc.vector.tensor_copy(ht_sb[:, jt, :], hp)

            mt_sb = big.tile([P, NDT, V], f32r, name="mt_sb", tag="mt_sb")
            for dt in range(NDT):
                for nt in range(NV):
                    mp = ps.tile(
                        [P, VW], f32, name=f"mp{dt}_{nt}",
                        tag=f"b{(dt * NV + nt) % 8}",
                    )
                    for jt in range(NDT):
                        nc.tensor.matmul(
                            mp,
                            ht_sb[:, jt, ts(dt, P)],
                            lwt_sb[:, jt, ds(nt * VW, VW)],
                            start=(jt == 0), stop=(jt == NDT - 1),
                        )
                    nc.vector.tensor_copy(mt_sb[:, dt, ds(nt * VW, VW)], mp)

            # ---- phase 3: logits = A Mt + 1 Lb^T, then log_softmax rows ----
            for lt in range(NLT):
                lp = []
                for nt in range(NV):
                    p_t = ps.tile(
                        [P, VW], f32, name=f"lp{lt}_{nt}",
                        tag=f"b{(lt * NV + nt) % 8}",
                    )
                    for kt in range(NDT):
                        nc.tensor.matmul(
                            p_t,
                            at_sb[:, kt, ts(lt, P)],
                            mt_sb[:, kt, ds(nt * VW, VW)],
                            start=(kt == 0), stop=False,
                        )
                    nc.tensor.matmul(
                        p_t, ones_mat, lb_sb[:, ds(nt * VW, VW)],
                        start=False, stop=True,
                    )
                    lp.append(p_t)

                mx0 = stat.tile([P, 1], f32, name="mx0", tag="mx0")
                nc.vector.reduce_max(mx0, lp[0], axis=X)
                mx1 = stat.tile([P, 1], f32, name="mx1", tag="mx1")
                nc.vector.reduce_max(mx1, lp[1], axis=X)
                nmx = stat.tile([P, 1], f32, name="nmx", tag="nmx")
                nc.vector.tensor_tensor(nmx, mx0, mx1, op=mybir.AluOpType.max)
                nc.vector.tensor_scalar_mul(nmx, nmx, -1.0)

                esc = work.tile([P, V], f32, name="esc", tag="esc")
                se0 = stat.tile([P, 1], f32, name="se0", tag="se0")
                se1 = stat.tile([P, 1], f32, name="se1", tag="se1")
                nc.scalar.activation(
                    esc[:, 0:VW], lp[0], EXP, bias=nmx, scale=1.0, accum_out=se0
                )
                nc.scalar.activation(
                    esc[:, VW:V], lp[1], EXP, bias=nmx, scale=1.0, accum_out=se1
                )
                sume = stat.tile([P, 1], f32, name="sume", tag="sume")
                nc.vector.tensor_add(sume, se0, se1)
                lns = stat.tile([P, 1], f32, name="lns", tag="lns")
                nc.scalar.activation(lns, sume, LN)
                shift = stat.tile([P, 1], f32, name="shift", tag="shift")
                nc.vector.tensor_tensor(
                    shift, nmx, lns, op=mybir.AluOpType.subtract
                )

                out_sb = work.tile([P, V], f32, name="out_sb", tag="out_sb")
                nc.scalar.activation(
                    out_sb[:, 0:VW], lp[0], IDENT, bias=shift, scale=1.0
                )
                nc.vector.tensor_scalar_add(out_sb[:, VW:V], lp[1], shift)
                nc.sync.dma_start(out_dram[lt], out_sb)

    nc.compile()
    return nc


def _get_nc():
    global _CACHED_NC
    if _CACHED_NC is None:
        _CACHED_NC = _build_nc()
    return _CACHED_NC


def _make_in_maps(x, Ww, Wb, Lw, Lb):
    x = np.asarray(x, dtype=np.float32)
    Ww = np.asarray(Ww, dtype=np.float32)
    Wb = np.asarray(Wb, dtype=np.float32)
    Lw = np.asarray(Lw, dtype=np.float32)
    Lb = np.asarray(Lb, dtype=np.float32)

    wwt = np.ascontiguousarray(
        Ww.T.reshape(NDT, P, D).transpose(1, 0, 2)
    )  # (P, NDT, D)
    lwt = np.ascontiguousarray(
        Lw.T.reshape(NDT, P, V).transpose(1, 0, 2)
    )  # (P, NDT, V)
    wb = np.ascontiguousarray(Wb.reshape(1, D))
    lb = np.zeros((P, V), np.float32)
    lb[0, :] = Lb

    in_maps = []
    for core in range(NCORES):
        b, h = core // 2, core % 2
        a_b = np.ascontiguousarray(x[:, b, :])                 # (L, D)
        a_full = a_b.reshape(NKT, P, D)
        at = np.ascontiguousarray(
            a_b[h * LC:(h + 1) * LC, :].T.reshape(NDT, P, LC).transpose(1, 0, 2)
        )  # (P, NDT, LC)
        in_maps.append({
            "a_full": np.ascontiguousarray(a_full),
            "at": at,
            "wwt": wwt,
            "lwt": lwt,
            "wb": wb,
            "lb": lb,
            "ones_col": _ONES_COL,
            "ones_mat": _ONES_MAT,
        })
    return in_maps


def kernel(x, Ww, Wb, Lw, Lb, _trace=False):
    from concourse.bass_utils import run_bass_kernel_spmd

    nc = _get_nc()
    in_maps = _make_in_maps(x, Ww, Wb, Lw, Lb)
    res = run_bass_kernel_spmd(
        nc, in_maps, core_ids=list(range(NCORES)), trace=_trace
    )
    out = np.empty((L, B, V), np.float32)
    for core in range(NCORES):
        b, h = core // 2, core % 2
        out[h * LC:(h + 1) * LC, b, :] = (
            res.results[core]["out"].reshape(LC, V)
        )
    if _trace:
        kernel._last_results = res
    return out


# revision 12
# speedup vs baseline: 1.5035x; 1.0135x over previous
"""Trainium2 Bass kernel for CTCDecoder-like module.

Reference computes (per batch b, with A = x[:, b, :] of shape (L, D)):
    wx     = A @ Ww^T + Wb
    scores = A @ wx^T                       # (L, L) -- never materialized here
    y      = scores @ A
    logits = y @ Lw^T + Lb
    out    = log_softmax(logits, axis=-1)

Algebraic collapse used by this kernel (exact in real arithmetic):
    scores = A Ww A^T + (A Wb) 1^T
    y      = A Ww (A^T A) + (A Wb)(1^T A) = A (Ww G + Wb c^T) = A H
    logits = A (H Lw^T) + 1 Lb^T = A Mt + 1 Lb^T
with G = A^T A (D x D Gram), c = A^T 1 (column sums), H = Ww G + Wb c^T,
Mt = H Lw^T (D x V).  The (L, L) score matrix is never formed.

Sharding: 8 cores = 4 batches x 2 halves of L.  Each core computes its
batch's Gram redundantly (pure SPMD, no collectives), then its own
2048-token logits + log_softmax.  All matmuls run as float32r (full-rate
fp32 streaming on TRN2 for free dims >= 256), accumulated in fp32 PSUM.
"""

import numpy as np

L, B, D, V = 4096, 4, 512, 1000
P = 128
NCORES = 8
LC = L // 2            # tokens per core
NKT = L // P           # 32 k-tiles over full L (Gram)
NLT = LC // P          # 16 l-tiles per core
NDT = D // P           # 4 d-tiles
NV = 2
VW = V // NV           # 500

_CACHED_NC = None
_ONES_COL = np.ones((P, 1), np.float32)
_ONES_MAT = np.ones((P, P), np.float32)
_IDENT = np.eye(P, dtype=np.float32)


def _build_nc():
    import concourse.bass as bass
    import concourse.tile as tile
    import concourse.mybir as mybir
    from concourse import bacc
    from concourse.bass import ds, ts

    f32 = mybir.dt.float32
    f32r = mybir.dt.float32r
    bf16 = mybir.dt.bfloat16
    EXP = mybir.ActivationFunctionType.Exp
    LN = mybir.ActivationFunctionType.Ln
    IDENT = mybir.ActivationFunctionType.Identity
    X = mybir.AxisListType.X

    # Serve Exp/Ln/Identity/Copy from the single act-func table that
    # contains them all ("natural_log_exp_and_others"), instead of letting
    # the table-load pass bounce between per-func tables (1283 ns per
    # reload on ScalarE).  Order (and thus act_func_set_id) is preserved;
    # only the advertised contents of the other tables are hidden.
    import concourse.bacc as bacc_mod
    from concourse.hw_specs import get_activation_tables

    def _pinned_tables(arch, _orig=get_activation_tables):
        tables = _orig(arch)
        keep = "natural_log_exp_and_others"
        if keep in tables:
            tables = {
                name: (funcs if name == keep else set())
                for name, funcs in tables.items()
            }
        return tables

    bacc_mod.get_activation_tables = _pinned_tables

    nc = bacc.Bacc("TRN2", target_bir_lowering=False, debug=False)

    a_dram = nc.dram_tensor("a_full", (NKT, P, D), f32r, kind="ExternalInput")
    at_dram = nc.dram_tensor("at", (P, NDT, LC), f32r, kind="ExternalInput")
    wwt_dram = nc.dram_tensor("wwt", (P, NDT, D), f32r, kind="ExternalInput")
    lwt_dram = nc.dram_tensor("lwt", (P, NDT, V), f32r, kind="ExternalInput")
    wb_dram = nc.dram_tensor("wb", (1, D), f32r, kind="ExternalInput")
    lb_dram = nc.dram_tensor("lb", (P, V), f32r, kind="ExternalInput")
    onesc_dram = nc.dram_tensor("ones_col", (P, 1), f32r, kind="ExternalInput")
    onesr_dram = nc.dram_tensor("ones_mat", (P, P), f32r, kind="ExternalInput")
    ident_dram = nc.dram_tensor("ident", (P, P), f32r, kind="ExternalInput")
    out_dram = nc.dram_tensor("out", (NLT, P, V), f32, kind="ExternalOutput")

    with tile.TileContext(nc) as tc:
        with (
            tc.tile_pool(name="const", bufs=1) as const,
            tc.tile_pool(name="big", bufs=1) as big,
            tc.tile_pool(name="astream", bufs=8) as astream,
            tc.tile_pool(name="work", bufs=3) as work,
            tc.tile_pool(name="stat", bufs=4) as stat,
            tc.tile_pool(name="ps", bufs=1, space="PSUM") as ps,
        ):
            # ---- tiny constants first (ones_col gates the first ct matmul) ----
            ones_col = const.tile([P, 1], f32r, name="ones_col", tag="ones_col")
            nc.sync.dma_start(ones_col, onesc_dram[:])
            ident_sb = const.tile([P, P], f32r, name="ident_sb", tag="ident_sb")
            ones_mat = const.tile([P, P], f32r, name="ones_mat", tag="ones_mat")
            wb_sb = const.tile([1, D], f32r, name="wb_sb", tag="wb_sb")
            lb_sb = const.tile([P, V], f32r, name="lb_sb", tag="lb_sb")
            at_sb = big.tile([P, NDT, LC], f32r, name="at_sb", tag="at_sb")
            wwt_sb = big.tile([P, NDT, D], f32r, name="wwt_sb", tag="wwt_sb")
            lwt_sb = big.tile([P, NDT, V], f32r, name="lwt_sb", tag="lwt_sb")

            # ---- phase 1: Gram G = A^T A and c^T = 1^T A over full L ----
            g_ps = [
                ps.tile([P, D], f32, name=f"g_ps{mt}", tag=f"b{mt}")
                for mt in range(NDT)
            ]
            ct_ps = ps.tile([1, D], f32, name="ct_ps", tag="b4")
            acc = big.tile([P, D], f32, name="acc", tag="acc")
            nc.vector.memset(acc, 0.0)
            for kt in range(NKT):
                a_t = astream.tile([P, D], f32r, name="a_t", tag="a_t")
                nc.sync.dma_start(a_t, a_dram[kt])
                first, last = kt == 0, kt == NKT - 1
                for mt in range(NDT):
                    n0 = (mt if mt < 3 else 2) * P
                    nc.tensor.matmul(
                        g_ps[mt][:, n0:D], a_t[:, ts(mt, P)], a_t[:, n0:D],
                        start=first, stop=last,
                    )
                nc.vector.tensor_add(acc, acc, a_t.bitcast(f32))
            acc_r = big.tile([P, D], f32r, name="acc_r", tag="acc_r")
            nc.vector.tensor_copy(acc_r, acc)
            nc.tensor.matmul(ct_ps, ones_col, acc_r, start=True, stop=True)

            # resident operands stream in while the PE chews on the Gram
            nc.sync.dma_start(wb_sb, wb_dram[:])
            nc.sync.dma_start(ident_sb, ident_dram[:])
            nc.sync.dma_start(wwt_sb, wwt_dram[:])
            nc.sync.dma_start(lb_sb, lb_dram[:])
            nc.sync.dma_start(ones_mat, onesr_dram[:])
            nc.sync.dma_start(lwt_sb, lwt_dram[:])
            nc.sync.dma_start(at_sb, at_dram[:])

            g_sb = big.tile([P, NDT, D], f32r, name="g_sb", tag="g_sb")
            for mt in range(NDT):
                n0 = (mt if mt < 3 else 2) * P
                nc.vector.tensor_copy(g_sb[:, mt, n0:D], g_ps[mt][:, n0:D])
            # G is symmetric: lower blocks (i,j) are transposes of stored (j,i)
            for idx, (i, j) in enumerate([(1, 0), (2, 0), (3, 0), (2, 1), (3, 1)]):
                tp = ps.tile([P, P], f32r, name=f"tp{i}{j}", tag=f"b{5 + idx % 3}")
                nc.tensor.transpose(tp, g_sb[:, j, ts(i, P)], ident_sb)
                nc.vector.tensor_copy(g_sb[:, i, ts(j, P)], tp)
            c_row = big.tile([1, D], f32r, name="c_row", tag="c_row")
            nc.vector.tensor_copy(c_row, ct_ps)

            # ---- phase 2: Ht = G Ww^T + c (x) Wb ; Mt = Ht^T Lw^T ----
            ht_sb = big.tile([P, NDT, D], f32r, name="ht_sb", tag="ht_sb")
            for jt in range(NDT):
                hp = ps.tile([P, D], f32, name=f"hp{jt}", tag=f"b{5 + jt % 3}")
                for kt in range(NDT):
                    nc.tensor.matmul(
                        hp, g_sb[:, kt, ts(jt, P)], wwt_sb[:, kt, :],
                        start=(kt == 0), stop=False,
                    )
                nc.tensor.matmul(
                    hp, c_row[:, ts(jt, P)], wb_sb, start=False, stop=True,
                )
                nc.vector.tensor_copy(ht_sb[:, jt, :], hp)

            mt_sb = big.tile([P, NDT, V], f32r, name="mt_sb", tag="mt_sb")
            for dt in range(NDT):
                for nt in range(NV):
                    mp = ps.tile(
                        [P, VW], f32, name=f"mp{dt}_{nt}",
                        tag=f"b{(dt * NV + nt) % 8}",
                    )
                    for jt in range(NDT):
                        nc.tensor.matmul(
                            mp,
                            ht_sb[:, jt, ts(dt, P)],
                            lwt_sb[:, jt, ds(nt * VW, VW)],
                            start=(jt == 0), stop=(jt == NDT - 1),
                        )
                    nc.vector.tensor_copy(mt_sb[:, dt, ds(nt * VW, VW)], mp)

            # ---- phase 3: logits = A Mt + 1 Lb^T, then log_softmax rows ----
            for lt in range(NLT):
                lp = []
                for nt in range(NV):
                    p_t = ps.tile(
                        [P, VW], f32, name=f"lp{lt}_{nt}",
                        tag=f"b{(lt * NV + nt) % 8}",
                    )
                    for kt in range(NDT):
                        nc.tensor.matmul(
                            p_t,
                            at_sb[:, kt, ts(lt, P)],
                            mt_sb[:, kt, ds(nt * VW, VW)],
                            start=(kt == 0), stop=False,
                        )
                    nc.tensor.matmul(
                        p_t, ones_mat, lb_sb[:, ds(nt * VW, VW)],
                        start=False, stop=True,
                    )
                    lp.append(p_t)

                mx0 = stat.tile([P, 1], f32, name="mx0", tag="mx0")
                nc.vector.reduce_max(mx0, lp[0], axis=X)
                mx1 = stat.tile([P, 1], f32, name="mx1", tag="mx1")
                nc.vector.reduce_max(mx1, lp[1], axis=X)
                nmx = stat.tile([P, 1], f32, name="nmx", tag="nmx")
                nc.vector.tensor_tensor(nmx, mx0, mx1, op=mybir.AluOpType.max)
                nc.vector.tensor_scalar_mul(nmx, nmx, -1.0)

                esc = work.tile([P, V], f32, name="esc", tag="esc")
                se0 = stat.tile([P, 1], f32, name="se0", tag="se0")
                se1 = stat.tile([P, 1], f32, name="se1", tag="se1")
                nc.scalar.activation(
                    esc[:, 0:VW], lp[0], EXP, bias=nmx, scale=1.0, accum_out=se0
                )
                nc.scalar.activation(
                    esc[:, VW:V], lp[1], EXP, bias=nmx, scale=1.0, accum_out=se1
                )
                sume = stat.tile([P, 1], f32, name="sume", tag="sume")
                nc.vector.tensor_add(sume, se0, se1)
                lns = stat.tile([P, 1], f32, name="lns", tag="lns")
                nc.scalar.activation(lns, sume, LN)
                shift = stat.tile([P, 1], f32, name="shift", tag="shift")
                nc.vector.tensor_tensor(
                    shift, nmx, lns, op=mybir.AluOpType.subtract
                )

                out_sb = work.tile([P, V], f32, name="out_sb", tag="out_sb")
                nc.scalar.activation(
                    out_sb[:, 0:VW], lp[0], IDENT, bias=shift, scale=1.0
                )
                nc.vector.tensor_scalar_add(out_sb[:, VW:V], lp[1], shift)
                nc.sync.dma_start(out_dram[lt], out_sb)

    nc.compile()
    return nc


def _get_nc():
    global _CACHED_NC
    if _CACHED_NC is None:
        _CACHED_NC = _build_nc()
    return _CACHED_NC


def _make_in_maps(x, Ww, Wb, Lw, Lb):
    x = np.asarray(x, dtype=np.float32)
    Ww = np.asarray(Ww, dtype=np.float32)
    Wb = np.asarray(Wb, dtype=np.float32)
    Lw = np.asarray(Lw, dtype=np.float32)
    Lb = np.asarray(Lb, dtype=np.float32)

    wwt = np.ascontiguousarray(
        Ww.T.reshape(NDT, P, D).transpose(1, 0, 2)
    )  # (P, NDT, D)
    lwt = np.ascontiguousarray(
        Lw.T.reshape(NDT, P, V).transpose(1, 0, 2)
    )  # (P, NDT, V)
    wb = np.ascontiguousarray(Wb.reshape(1, D))
    lb = np.zeros((P, V), np.float32)
    lb[0, :] = Lb

    in_maps = []
    for core in range(NCORES):
        b, h = core // 2, core % 2
        a_b = np.ascontiguousarray(x[:, b, :])                 # (L, D)
        a_full = a_b.reshape(NKT, P, D)
        at = np.ascontiguousarray(
            a_b[h * LC:(h + 1) * LC, :].T.reshape(NDT, P, LC).transpose(1, 0, 2)
        )  # (P, NDT, LC)
        in_maps.append({
            "a_full": np.ascontiguousarray(a_full),
            "at": at,
            "wwt": wwt,
            "lwt": lwt,
            "wb": wb,
            "lb": lb,
            "ones_col": _ONES_COL,
            "ones_mat": _ONES_MAT,
            "ident": _IDENT,
        })
    return in_maps


def kernel(x, Ww, Wb, Lw, Lb, _trace=False):
    from concourse.bass_utils import run_bass_kernel_spmd

    nc = _get_nc()
    in_maps = _make_in_maps(x, Ww, Wb, Lw, Lb)
    res = run_bass_kernel_spmd(
        nc, in_maps, core_ids=list(range(NCORES)), trace=_trace
    )
    out = np.empty((L, B, V), np.float32)
    for core in range(NCORES):
        b, h = core // 2, core % 2
        out[h * LC:(h + 1) * LC, b, :] = (
            res.results[core]["out"].reshape(LC, V)
        )
    if _trace:
        kernel._last_results = res
    return out
